# revision 41
# baseline (speedup 1.0000x reference)
"""Distributed Chebyshev SpMM kernel for 8 Trainium2 NeuronCores.

acc = sum_k coeffs[k] * T_k(L) @ X with T_k = 2 L T_{k-1} - T_{k-2} over a
sparse 50000-node / 800000-edge graph, feature dim 128, 30 coefficients.

Strategy: row-shard nodes across 8 cores. Per Chebyshev step each core
dma_gathers T_{k-1}[col] rows (bf16, 256B) for its ~100K edges from an HBM
copy of the full T_{k-1}, spread over all 4 SWDGE queues (the per-queue
descriptor ring is the gather bottleneck), and segment-reduces on the
TensorEngine: gathered tile is the bf16 stationary operand, a
host-precomputed 2*val*onehot bf16 mask is the moving operand, accumulating
fp32 output rows in PSUM. The recurrence/accumulator math stays fp32 in
SBUF (in-place two-slab update).

The gather source is split into two buffers by source-local row
(col % RPC < HALF1), each rebuilt by its own AllGather into Shared,
double-buffered DRAM: the half-1 AllGather launches as soon as windows
0..7 of the new T_k slice are stored and overlaps the rest of the
iteration; only the (smaller) half-2 AllGather sits on the critical path
between iterations. Masks/indices are iteration-invariant and loaded to
SBUF once; gather indices are int16 (8*4096 rows just fits). The edge
packer uses span cap 16 for the dense half-1 segments and 32 for the
sparser half-2 segments (~95% tile fill).
"""
import sys
sys.path.insert(0, "/opt/trn_rl_repo")
import os
import numpy as np

_SKIP_AG = bool(int(os.environ.get("K_SKIP_AG", "0")))
_SKIP_GATHER = bool(int(os.environ.get("K_SKIP_GATHER", "0")))
_SKIP_MM = bool(int(os.environ.get("K_SKIP_MM", "0")))
_SKIP_EPI = bool(int(os.environ.get("K_SKIP_EPI", "0")))
_LOCAL_XBUF = bool(int(os.environ.get("K_LOCAL_XBUF", "0")))


N = 50000
D = 128
NNZ = 800000
M = 30
CORES = 8
RPC = N // CORES          # 6250
W = 512
NW = (RPC + W - 1) // W   # 13 (12x512 + 106)
TILE = 128
SPAN_S = (16, 32)  # packer span cap per segment kind (dense s0 / sparse s1)
# Gather-source split: edges are bucketed by whether their source node's
# LOCAL row (col % RPC) falls in [0, HALF1) or [HALF1, RPC). Each bucket has
# its own AllGather'd buffer (Xbuf1: CORES*HALF1 rows, Xbuf2: CORES*HALF2),
# so the T_k slice halves can be AllGather'd as soon as windows 0..NW1-1 /
# NW1..NW-1 finish.  8*4096-1 = 32767 just fits int16 gather indices.
HALF1 = 4096
HALF2 = RPC - HALF1       # 2154
NW1 = HALF1 // W          # 8 windows in half 1


def build_plans(rows, cols, vals):
    """Returns (shared_segs, per_core) where
    shared_segs: list over segments of dict(w, s, ntiles, idx_off,
                 r0s[ntiles], spans[ntiles])
    per_core: list of dict(idxs int16 [16, L/16], masks f32 [128, Ttot*SPAN])
    """
    rows = np.asarray(rows).astype(np.int64)
    cols = np.asarray(cols).astype(np.int64)
    vals = np.asarray(vals).astype(np.float32)

    # per-core sorted edge lists per (w, s); s buckets by col % RPC vs HALF1,
    # and edge cols are pre-mapped to gather indices into Xbuf1/Xbuf2.
    core_seg_edges = [[] for _ in range(CORES)]  # [(er, ec, ev)] per segment
    for c in range(CORES):
        r0c = c * RPC
        sel = (rows >= r0c) & (rows < r0c + RPC)
        er_all = rows[sel] - r0c
        ec_all = cols[sel]
        ev_all = vals[sel]
        g_all = ec_all // RPC
        rr_all = ec_all % RPC
        s_all = rr_all >= HALF1
        gi_all = np.where(s_all, g_all * HALF2 + (rr_all - HALF1),
                          g_all * HALF1 + rr_all)
        for w in range(NW):
            rlo = w * W
            rhi = min(rlo + W, RPC)
            inw = (er_all >= rlo) & (er_all < rhi)
            for s in range(2):
                m = inw & (s_all == bool(s))
                er = er_all[m] - rlo
                ec = gi_all[m]
                ev = ev_all[m]
                o = np.argsort(er, kind="stable")
                core_seg_edges[c].append((er[o], ec[o], ev[o]))

    nseg = NW * 2
    shared_segs = []
    per_core_tiles = [[] for _ in range(CORES)]  # (idx128, rw128, val128) per tile
    tile_moffs = []  # mask column offset per global tile
    idx_off = 0
    moff = 0
    for si in range(nseg):
        w, s = divmod(si, 2)
        wsize = min(W, RPC - w * W)
        span_cap = SPAN_S[s]
        # Joint greedy schedule: r0_t = min over cores of next pending row;
        # each core then takes up to 128 edges with rows < r0_t + span.
        # Feasible by construction for every core.
        segs_e = [core_seg_edges[c][si] for c in range(CORES)]
        pos = [0] * CORES
        nes = [len(e[0]) for e in segs_e]
        r0s, spans = [], []
        takes = []  # per tile: list of (core_pos, take)
        prev = 0
        while any(pos[c] < nes[c] for c in range(CORES)):
            nextrow = min(
                (int(segs_e[c][0][pos[c]]) for c in range(CORES)
                 if pos[c] < nes[c]),
            )
            r0 = max(prev, min(nextrow, max(0, wsize - 1)))
            span = min(span_cap, wsize - r0)
            tile_takes = []
            for c in range(CORES):
                er = segs_e[c][0]
                hi = np.searchsorted(er, r0 + span)
                take = int(min(TILE, hi - pos[c]))
                take = max(0, take)
                tile_takes.append((pos[c], take))
                pos[c] += take
            r0s.append(r0)
            spans.append(span)
            takes.append(tile_takes)
            prev = r0
            assert len(r0s) < 96, (si, len(r0s))
        ntiles = len(r0s)

        # pack each core
        for c in range(CORES):
            er, ec, ev = segs_e[c]
            for t in range(ntiles):
                r0, span = r0s[t], spans[t]
                p0, take = takes[t][c]
                idx_t = np.zeros(TILE, np.int64)
                rw_t = np.full(TILE, r0, np.int64)
                val_t = np.zeros(TILE, np.float32)
                if take > 0:
                    idx_t[:take] = ec[p0:p0 + take]
                    rw_t[:take] = er[p0:p0 + take]
                    val_t[:take] = 2.0 * ev[p0:p0 + take]
                    assert er[p0] >= r0, (c, si, t, er[p0], r0)
                    assert er[p0 + take - 1] < r0 + span
                per_core_tiles[c].append((idx_t, rw_t - r0, val_t))
            assert pos[c] == len(er), (c, si, pos[c], len(er))

        shared_segs.append(dict(w=w, s=s, ntiles=ntiles, idx_off=idx_off,
                                r0s=r0s, spans=spans, moff0=moff,
                                mstride=span_cap))
        tile_moffs.extend(moff + t * span_cap for t in range(ntiles))
        idx_off += ntiles * TILE
        moff += ntiles * span_cap

    L = idx_off
    MW = moff
    Ttot = L // TILE
    per_core = []
    for c in range(CORES):
        tiles = per_core_tiles[c]
        idx_flat = np.concatenate([t[0] for t in tiles])
        masks = np.zeros((TILE, MW), np.float32)
        for g, (idx_t, loc_t, val_t) in enumerate(tiles):
            masks[np.arange(TILE), tile_moffs[g] + loc_t] = val_t
        idxs = np.ascontiguousarray(np.tile(idx_flat.reshape(L // 16, 16).T.astype(np.int16), (8, 1)))
        per_core.append(dict(idxs=idxs, masks=masks))
    return shared_segs, per_core, Ttot, L, MW


def sim_core_spmm(shared_segs, core_data, xb):
    """Numpy sim of one SpMM: returns [128, RPC] feat-major = rows of 2*L@X.
    xb = (xb1, xb2): gather sources [CORES*HALF1, D] / [CORES*HALF2, D]."""
    out = np.zeros((D, RPC), np.float32)
    idxs = core_data["idxs"]
    masks = core_data["masks"]
    for seg in shared_segs:
        src = xb[seg["s"]]
        Lseg = seg["ntiles"] * TILE
        off = seg["idx_off"]
        j = np.arange(Lseg)
        unwrapped = idxs[(off + j) % 16, (off + j) // 16].astype(np.int64)
        G = src[unwrapped]
        for t in range(seg["ntiles"]):
            Gt = G[t * TILE:(t + 1) * TILE]
            r0, span = seg["r0s"][t], seg["spans"][t]
            mo = seg["moff0"] + t * seg["mstride"]
            mk = masks[:, mo: mo + span]
            out[:, seg["w"] * W + r0: seg["w"] * W + r0 + span] += Gt.T @ mk
    return out


from concourse import bass, mybir, bacc
from concourse import tile
from concourse.bass_utils import run_bass_kernel_spmd

F32 = mybir.dt.float32
BF16 = mybir.dt.bfloat16
I16 = mybir.dt.int16
ALU = mybir.AluOpType

GCHUNK = int(os.environ.get("K_GCHUNK", "36"))  # tiles per gather call
NQ = int(os.environ.get("K_NQ", "4"))      # SWDGE gather queues
GBUFS = int(os.environ.get("K_GBUFS", "6"))  # gather tile pool depth


def build_kernel(shared_segs, Ttot, L, MW, n_iters=M - 1):
    """One shared SPMD program; per-core variation via inputs only.

    Inputs : xfull1/xfull2 [8*HALF1/2, D] bf16 (block-permuted X),
             x0T [128,RPC] f32, masks [128,Ttot*SPAN] bf16,
             idxs [16, L//16] i16, coefb [128,M] f32, ident [128,128] f32
    Output : out [RPC, D] f32 (own slice of acc)
    """
    nc = bacc.Bacc(None, target_bir_lowering=False, debug=False,
                   num_swdge_queues=NQ)

    xfull1_e = nc.declare_dram_parameter("xfull1", [CORES * HALF1, D], BF16,
                                         isOutput=False)
    xfull2_e = nc.declare_dram_parameter("xfull2", [CORES * HALF2, D], BF16,
                                         isOutput=False)
    x0T_e = nc.declare_dram_parameter("x0T", [128, RPC], F32, isOutput=False)
    masks_e = nc.declare_dram_parameter("masks", [128, MW], BF16, isOutput=False)
    idxs_e = nc.declare_dram_parameter("idxs", [128, L // 16], I16, isOutput=False)
    coefb_e = nc.declare_dram_parameter("coefb", [128, M], F32, isOutput=False)
    ident_e = nc.declare_dram_parameter("ident", [128, 128], F32, isOutput=False)
    out_e = nc.declare_dram_parameter("out", [RPC, D], F32, isOutput=True)

    space = "Local" if _LOCAL_XBUF else "Shared"
    Xb1 = [nc.dram_tensor(f"Xbuf1{p}", [CORES * HALF1, D], BF16,
                          addr_space=space) for p in "ab"]
    Xb2 = [nc.dram_tensor(f"Xbuf2{p}", [CORES * HALF2, D], BF16,
                          addr_space=space) for p in "ab"]
    slice1 = nc.dram_tensor("slice1", [HALF1, D], BF16)
    slice2 = nc.dram_tensor("slice2", [HALF2, D], BF16)

    with tile.TileContext(nc) as tc:
        with (
            tc.tile_pool(name="const", bufs=1) as cpool,
            tc.tile_pool(name="state", bufs=1) as spool,
            tc.tile_pool(name="g", bufs=GBUFS) as gpool,
            tc.tile_pool(name="stage", bufs=2) as stpool,
            tc.tile_pool(name="psw", bufs=2, space="PSUM") as pswpool,
            tc.tile_pool(name="pst", bufs=2, space="PSUM") as pstpool,
        ):
            masks_sb = cpool.tile([128, MW], BF16)
            idxs_sb = cpool.tile([128, L // 16], I16)
            coefb_sb = cpool.tile([128, M], F32)
            ident_sb = cpool.tile([128, 128], F32)

            slabs = [spool.tile([128, RPC], F32, tag=f"slab{i}", name=f"slab{i}") for i in range(2)]
            acc = spool.tile([128, RPC], F32, tag="acc")

            # ---- prologue: load constants + init state ----
            nc.sync.dma_start(out=masks_sb[:, :], in_=masks_e[:, :])
            nc.sync.dma_start(out=idxs_sb[:, :], in_=idxs_e[:, :])
            nc.sync.dma_start(out=coefb_sb[:, :], in_=coefb_e[:, :])
            nc.sync.dma_start(out=ident_sb[:, :], in_=ident_e[:, :])
            nc.sync.dma_start(out=slabs[0][:, :], in_=x0T_e[:, :])
            # iteration k reads parity k&1; k=1 reads index 1
            nc.gpsimd.dma_start(out=Xb1[1][:, :], in_=xfull1_e[:, :])
            nc.gpsimd.dma_start(out=Xb2[1][:, :], in_=xfull2_e[:, :])
            # acc = c0 * T0
            nc.vector.tensor_scalar(
                acc[:, :], slabs[0][:, :], coefb_sb[:, 0:1], None, ALU.mult
            )

            def emit_windows(k, Tprev2, Tout, do_store):
                """One SpMM pass + per-window epilogue.
                do_store: store transposed T_k rows to slice1/2 and AllGather
                them into the parity-(k+1)&1 gather buffers.
                """
                qi = 0
                rd, wr = k & 1, (k + 1) & 1
                for w in range(NW):
                    wlo = w * W
                    wsize = min(W, RPC - wlo)
                    psw = pswpool.tile([128, W], F32)
                    nc.vector.memset(psw[:, :wsize], 0.0)
                    for s in (0, 1):
                        seg = shared_segs[2 * w + s]
                        src = (Xb1[rd] if s == 0 else Xb2[rd])[:, :]
                        nt_all = seg["ntiles"]
                        for c0 in range(0, nt_all, GCHUNK):
                            nt = min(GCHUNK, nt_all - c0)
                            G = gpool.tile([128, GCHUNK, TILE], BF16, tag="g", name="gt")
                            off = seg["idx_off"] + c0 * TILE
                            if not _SKIP_GATHER:
                                nc.gpsimd.dma_gather(
                                    G[:, :nt, :],
                                    src,
                                    idxs_sb[:, off // 16: off // 16 + nt * TILE // 16],
                                    nt * TILE,
                                    nt * TILE,
                                    TILE,
                                    single_packet=False,
                                    queue_num=qi % NQ,
                                )
                                qi += 1
                            for t in range(nt):
                                mo = seg["moff0"] + (c0 + t) * seg["mstride"]
                                r0 = seg["r0s"][c0 + t]
                                span = seg["spans"][c0 + t]
                                is_last = (
                                    s == 1 and c0 + nt == nt_all and t == nt - 1
                                )
                                if not _SKIP_MM:
                                    nc.tensor.matmul(
                                        psw[:, r0:r0 + span],
                                        G[:, t, :],
                                        masks_sb[:, mo: mo + span],
                                        start=False,
                                        stop=is_last,
                                        skip_group_check=True,
                                    )
                    # window epilogue
                    if _SKIP_EPI:
                        continue
                    if k == 1:
                        nc.vector.tensor_scalar(
                            Tout[:, wlo:wlo + wsize], psw[:, :wsize],
                            0.5, None, ALU.mult,
                        )
                    else:
                        nc.vector.scalar_tensor_tensor(
                            Tout[:, wlo:wlo + wsize], psw[:, :wsize], 1.0,
                            Tprev2[:, wlo:wlo + wsize], ALU.mult, ALU.subtract,
                        )
                    nc.vector.scalar_tensor_tensor(
                        acc[:, wlo:wlo + wsize], Tout[:, wlo:wlo + wsize],
                        coefb_sb[:, k:k + 1], acc[:, wlo:wlo + wsize],
                        ALU.mult, ALU.add,
                    )
                    if do_store:
                        if w < NW1:
                            emit_transposed_store(Tout, wlo, wsize, slice1,
                                                  wlo, dt=BF16)
                        else:
                            emit_transposed_store(Tout, wlo, wsize, slice2,
                                                  wlo - HALF1, dt=BF16)
                        if not _SKIP_AG and w == NW1 - 1:
                            nc.gpsimd.collective_compute(
                                "AllGather", ALU.bypass,
                                replica_groups=[list(range(CORES))],
                                ins=[slice1.ap().opt()],
                                outs=[Xb1[wr].ap().opt()],
                            )
                        if not _SKIP_AG and w == NW - 1:
                            nc.gpsimd.collective_compute(
                                "AllGather", ALU.bypass,
                                replica_groups=[list(range(CORES))],
                                ins=[slice2.ap().opt()],
                                outs=[Xb2[wr].ap().opt()],
                            )

            def emit_transposed_store(srcT, wlo, wsize, dest, dlo, dt=F32):
                """dest[dlo:dlo+wsize, :] = srcT[:, wlo:wlo+wsize].T"""
                nq = (wsize + 127) // 128
                pst = pstpool.tile([128, W], F32)
                stage = stpool.tile([128, W], dt)
                for q in range(nq):
                    qsz = min(128, wsize - q * 128)
                    nc.tensor.transpose(
                        pst[:qsz, q * 128:q * 128 + 128],
                        srcT[:, wlo + q * 128: wlo + q * 128 + qsz],
                        ident_sb[:, :],
                    )
                if wsize % 128 == 0:
                    nc.vector.tensor_copy(stage[:, :nq * 128], pst[:, :nq * 128])
                else:
                    nc.vector.tensor_copy(
                        stage[:wsize, :nq * 128], pst[:wsize, :nq * 128]
                    )
                if wsize % 128 == 0:
                    # dest row (dlo + q*128 + p) <- stage[p, q*128 + f]
                    dest_ap = dest[dlo:dlo + wsize, :].rearrange(
                        "(q p) f -> p q f", p=128
                    )
                    nc.sync.dma_start(out=dest_ap, in_=stage[:, :nq * 128])
                else:
                    # last partial window: single q, partial partitions
                    assert nq == 1
                    nc.sync.dma_start(
                        out=dest[dlo:dlo + wsize, :], in_=stage[:wsize, :D]
                    )

            for k in range(1, n_iters + 1):
                # In-place recurrence: T_k overwrites T_{k-2}'s slab; the
                # epilogue reads Tprev2[w] and writes Tout[w] elementwise.
                Tprev2 = slabs[k % 2]
                Tout = slabs[k % 2]
                emit_windows(k, Tprev2, Tout, do_store=k < n_iters)

            # epilogue: out = acc.T
            for w in range(NW):
                wlo = w * W
                wsize = min(W, RPC - wlo)
                emit_transposed_store(acc, wlo, wsize, out_e, wlo)

    return nc


def _make_in_maps(X, coeffs, per_core):
    bfnp = mybir.dt.np(BF16)
    ident = np.eye(128, dtype=np.float32)
    coefb = np.broadcast_to(np.asarray(coeffs, np.float32)[None, :], (128, M)).copy()
    X_bf = np.ascontiguousarray(X).astype(bfnp)
    xfull1 = np.ascontiguousarray(np.concatenate(
        [X_bf[g * RPC: g * RPC + HALF1] for g in range(CORES)]))
    xfull2 = np.ascontiguousarray(np.concatenate(
        [X_bf[g * RPC + HALF1: (g + 1) * RPC] for g in range(CORES)]))
    in_maps = []
    for c in range(CORES):
        x0T = np.ascontiguousarray(X[c * RPC:(c + 1) * RPC].T)
        in_maps.append({
            "xfull1": xfull1,
            "xfull2": xfull2,
            "x0T": x0T,
            "masks": per_core[c]["masks"].astype(bfnp),
            "idxs": per_core[c]["idxs"],
            "coefb": coefb,
            "ident": ident,
        })
    return in_maps


def kernel(rows, cols, vals, X, coeffs, _trace=False):
    rows = np.asarray(rows)
    cols = np.asarray(cols)
    vals = np.asarray(vals, np.float32)
    X = np.asarray(X, np.float32)
    coeffs = np.asarray(coeffs, np.float32)

    shared_segs, per_core, Ttot, L, MW = build_plans(rows, cols, vals)
    nc = build_kernel(shared_segs, Ttot, L, MW, n_iters=M - 1)
    nc.compile()
    in_maps = _make_in_maps(X, coeffs, per_core)
    res = run_bass_kernel_spmd(nc, in_maps, list(range(CORES)), trace=_trace)
    out = np.concatenate([res.results[c]["out"] for c in range(CORES)], axis=0)
    if _trace:
        kernel.last_results = res
    return out


def bench_exec_ns(rows, cols, vals, X, coeffs, reps=5):
    """Time on-device execution with device-resident inputs (excludes
    compile and H2D of the real inputs; fresh donated output buffers are
    staged untimed before each rep)."""
    import time
    import jax
    from jax.sharding import Mesh, PartitionSpec
    from jax.experimental.shard_map import shard_map
    from concourse import bass2jax
    from concourse.bass2jax import _bass_exec_p, partition_id_tensor
    import concourse.mybir as _mb

    rows = np.asarray(rows); cols = np.asarray(cols)
    vals = np.asarray(vals, np.float32); X = np.asarray(X, np.float32)
    coeffs = np.asarray(coeffs, np.float32)
    shared_segs, per_core, Ttot, L, MW = build_plans(rows, cols, vals)
    nc = build_kernel(shared_segs, Ttot, L, MW, n_iters=M - 1)
    nc.compile()
    in_maps = _make_in_maps(X, coeffs, per_core)

    bass2jax.install_neuronx_cc_hook()
    partition_name = nc.partition_id_tensor.name if nc.partition_id_tensor else None
    in_names, out_names, out_avals, zero_outs = [], [], [], []
    for alloc in nc.m.functions[0].allocations:
        if not isinstance(alloc, _mb.MemoryLocationSet):
            continue
        name = alloc.memorylocations[0].name
        if alloc.kind == "ExternalInput":
            if name != partition_name:
                in_names.append(name)
        elif alloc.kind == "ExternalOutput":
            out_names.append(name)
            shape = tuple(alloc.tensor_shape)
            dtype = _mb.dt.np(alloc.dtype)
            out_avals.append(jax.core.ShapedArray(shape, dtype))
            zero_outs.append(np.zeros(shape, dtype))
    n_params = len(in_names)
    n_outs = len(out_avals)
    in_names.extend(out_names)
    if partition_name is not None:
        in_names.append(partition_name)
    donate = tuple(range(n_params, n_params + n_outs))

    def _body(*args):
        operands = list(args)
        if partition_name is not None:
            operands.append(partition_id_tensor())
        return tuple(_bass_exec_p.bind(
            *operands, out_avals=tuple(out_avals), in_names=tuple(in_names),
            out_names=tuple(out_names), lowering_input_output_aliases=(),
            sim_require_finite=False, sim_require_nnan=False, nc=nc))

    devices = jax.devices()[:CORES]
    mesh = Mesh(np.asarray(devices), ("core",))
    sharded = jax.jit(
        shard_map(_body, mesh=mesh,
                  in_specs=(PartitionSpec("core"),) * (n_params + n_outs),
                  out_specs=(PartitionSpec("core"),) * n_outs,
                  check_rep=False),
        donate_argnums=donate, keep_unused=True)
    per_core_in = [[np.asarray(m[nm]) for nm in in_names[:n_params]] for m in in_maps]
    concat_in = [np.concatenate([per_core_in[c][i] for c in range(CORES)], axis=0)
                 for i in range(n_params)]
    sharding = jax.sharding.NamedSharding(mesh, PartitionSpec("core"))
    dev_in = [jax.device_put(a, sharding) for a in concat_in]

    def fresh_zeros():
        return [jax.device_put(
            np.zeros((CORES * z.shape[0], *z.shape[1:]), z.dtype), sharding)
            for z in zero_outs]

    # warmup (compiles)
    outs = sharded(*dev_in, *fresh_zeros())
    jax.block_until_ready(outs)
    times = []
    for _ in range(reps):
        zs = fresh_zeros()
        jax.block_until_ready(zs)
        t0 = time.perf_counter()
        outs = sharded(*dev_in, *zs)
        jax.block_until_ready(outs)
        times.append(time.perf_counter() - t0)
    return int(min(times) * 1e9), [int(t * 1e9) for t in times]



# revision 52
# speedup vs baseline: 1.2937x; 1.2937x over previous
"""Distributed Chebyshev SpMM kernel for 8 Trainium2 NeuronCores.

acc = sum_k coeffs[k] * T_k(L) @ X with T_k = 2 L T_{k-1} - T_{k-2} over a
sparse 50000-node / 800000-edge graph, feature dim 128, 30 coefficients.

Strategy: row-shard nodes across 8 cores. Per Chebyshev step each core
dma_gathers T_{k-1}[col] rows (bf16, 256B) for its ~100K edges from an HBM
copy of the full T_{k-1}, spread over all 4 SWDGE queues (the per-queue
descriptor ring is the gather bottleneck), and segment-reduces on the
TensorEngine: gathered tile is the bf16 stationary operand, a
host-precomputed 2*val*onehot bf16 mask is the moving operand, accumulating
fp32 output rows in PSUM. The recurrence/accumulator math stays fp32 in
SBUF (in-place two-slab update).

The new T_k slice is transposed-stored to HBM per window and one
AllGather per iteration rebuilds the full bf16 gather source (Shared
address space for the fast collective path). Masks/indices are
iteration-invariant and loaded to SBUF once; gather indices are int16,
so edges are split into two col-range streams (<25000 / >=25000).
"""
import sys
sys.path.insert(0, "/opt/trn_rl_repo")
import os
import numpy as np

_SKIP_AG = bool(int(os.environ.get("K_SKIP_AG", "0")))
_SKIP_GATHER = bool(int(os.environ.get("K_SKIP_GATHER", "0")))
_SKIP_MM = bool(int(os.environ.get("K_SKIP_MM", "0")))
_SKIP_EPI = bool(int(os.environ.get("K_SKIP_EPI", "0")))
_LOCAL_XBUF = bool(int(os.environ.get("K_LOCAL_XBUF", "0")))


N = 50000
D = 128
NNZ = 800000
M = 30
CORES = 8
RPC = N // CORES          # 6250
W = 512
NW = (RPC + W - 1) // W   # 13 (12x512 + 106)
TILE = 128
_SPAN = int(os.environ.get("K_SPAN", "16"))
SPAN_S = (_SPAN, _SPAN)  # packer span cap per segment kind
# Edges are bucketed by global source column half (col < 25000): gather
# indices stay within int16 range and each half is a contiguous row range
# of the single AllGather'd Xbuf.
COLSPLIT = 25000


def build_plans(rows, cols, vals):
    """Returns (shared_segs, per_core) where
    shared_segs: list over segments of dict(w, s, ntiles, idx_off,
                 r0s[ntiles], spans[ntiles])
    per_core: list of dict(idxs int16 [16, L/16], masks f32 [128, Ttot*SPAN])
    """
    rows = np.asarray(rows).astype(np.int64)
    cols = np.asarray(cols).astype(np.int64)
    vals = np.asarray(vals).astype(np.float32)

    # per-core sorted edge lists per (w, s); s buckets by global col half,
    # cols pre-mapped to gather indices into the matching Xbuf half.
    core_seg_edges = [[] for _ in range(CORES)]  # [(er, ec, ev)] per segment
    for c in range(CORES):
        r0c = c * RPC
        sel = (rows >= r0c) & (rows < r0c + RPC)
        er_all = rows[sel] - r0c
        ec_all = cols[sel]
        ev_all = vals[sel]
        s_all = ec_all >= COLSPLIT
        gi_all = np.where(s_all, ec_all - COLSPLIT, ec_all)
        for w in range(NW):
            rlo = w * W
            rhi = min(rlo + W, RPC)
            inw = (er_all >= rlo) & (er_all < rhi)
            for s in range(2):
                m = inw & (s_all == bool(s))
                er = er_all[m] - rlo
                ec = gi_all[m]
                ev = ev_all[m]
                o = np.argsort(er, kind="stable")
                core_seg_edges[c].append((er[o], ec[o], ev[o]))

    nseg = NW * 2
    shared_segs = []
    per_core_tiles = [[] for _ in range(CORES)]  # (idx128, rw128, val128) per tile
    tile_moffs = []  # mask column offset per global tile
    idx_off = 0
    moff = 0
    for si in range(nseg):
        w, s = divmod(si, 2)
        wsize = min(W, RPC - w * W)
        span_cap = SPAN_S[s]
        # Joint greedy schedule: r0_t = min over cores of next pending row;
        # each core then takes up to 128 edges with rows < r0_t + span.
        # Feasible by construction for every core.
        segs_e = [core_seg_edges[c][si] for c in range(CORES)]
        pos = [0] * CORES
        nes = [len(e[0]) for e in segs_e]
        r0s, spans = [], []
        takes = []  # per tile: list of (core_pos, take)
        prev = 0
        while any(pos[c] < nes[c] for c in range(CORES)):
            nextrow = min(
                (int(segs_e[c][0][pos[c]]) for c in range(CORES)
                 if pos[c] < nes[c]),
            )
            r0 = max(prev, min(nextrow, max(0, wsize - 1)))
            span = min(span_cap, wsize - r0)
            tile_takes = []
            for c in range(CORES):
                er = segs_e[c][0]
                hi = np.searchsorted(er, r0 + span)
                take = int(min(TILE, hi - pos[c]))
                take = max(0, take)
                tile_takes.append((pos[c], take))
                pos[c] += take
            r0s.append(r0)
            spans.append(span)
            takes.append(tile_takes)
            prev = r0
            assert len(r0s) < 96, (si, len(r0s))
        ntiles = len(r0s)

        # pack each core
        for c in range(CORES):
            er, ec, ev = segs_e[c]
            for t in range(ntiles):
                r0, span = r0s[t], spans[t]
                p0, take = takes[t][c]
                idx_t = np.zeros(TILE, np.int64)
                rw_t = np.full(TILE, r0, np.int64)
                val_t = np.zeros(TILE, np.float32)
                if take > 0:
                    idx_t[:take] = ec[p0:p0 + take]
                    rw_t[:take] = er[p0:p0 + take]
                    val_t[:take] = 2.0 * ev[p0:p0 + take]
                    assert er[p0] >= r0, (c, si, t, er[p0], r0)
                    assert er[p0 + take - 1] < r0 + span
                per_core_tiles[c].append((idx_t, rw_t - r0, val_t))
            assert pos[c] == len(er), (c, si, pos[c], len(er))

        shared_segs.append(dict(w=w, s=s, ntiles=ntiles, idx_off=idx_off,
                                r0s=r0s, spans=spans, moff0=moff,
                                mstride=span_cap))
        tile_moffs.extend(moff + t * span_cap for t in range(ntiles))
        idx_off += ntiles * TILE
        moff += ntiles * span_cap

    L = idx_off
    MW = moff
    Ttot = L // TILE
    per_core = []
    for c in range(CORES):
        tiles = per_core_tiles[c]
        idx_flat = np.concatenate([t[0] for t in tiles])
        masks = np.zeros((TILE, MW), np.float32)
        for g, (idx_t, loc_t, val_t) in enumerate(tiles):
            masks[np.arange(TILE), tile_moffs[g] + loc_t] = val_t
        idxs = np.ascontiguousarray(np.tile(idx_flat.reshape(L // 16, 16).T.astype(np.int16), (8, 1)))
        per_core.append(dict(idxs=idxs, masks=masks))
    return shared_segs, per_core, Ttot, L, MW


def sim_core_spmm(shared_segs, core_data, xb):
    """Numpy sim of one SpMM: returns [128, RPC] feat-major = rows of 2*L@X.
    xb = (X[:25000], X[25000:]): the two gather-source halves."""
    out = np.zeros((D, RPC), np.float32)
    idxs = core_data["idxs"]
    masks = core_data["masks"]
    for seg in shared_segs:
        src = xb[seg["s"]]
        Lseg = seg["ntiles"] * TILE
        off = seg["idx_off"]
        j = np.arange(Lseg)
        unwrapped = idxs[(off + j) % 16, (off + j) // 16].astype(np.int64)
        G = src[unwrapped]
        for t in range(seg["ntiles"]):
            Gt = G[t * TILE:(t + 1) * TILE]
            r0, span = seg["r0s"][t], seg["spans"][t]
            mo = seg["moff0"] + t * seg["mstride"]
            mk = masks[:, mo: mo + span]
            out[:, seg["w"] * W + r0: seg["w"] * W + r0 + span] += Gt.T @ mk
    return out


from concourse import bass, mybir, bacc
from concourse import tile
from concourse.bass_utils import run_bass_kernel_spmd

F32 = mybir.dt.float32
BF16 = mybir.dt.bfloat16
I16 = mybir.dt.int16
ALU = mybir.AluOpType

GCHUNK = int(os.environ.get("K_GCHUNK", "18"))  # tiles per gather call
NQ = int(os.environ.get("K_NQ", "4"))      # SWDGE gather queues
GBUFS = int(os.environ.get("K_GBUFS", "8"))  # gather tile pool depth


def build_kernel(shared_segs, Ttot, L, MW, n_iters=M - 1):
    """One shared SPMD program; per-core variation via inputs only.

    Inputs : xfull [N, D] bf16,
             x0T [128,RPC] f32, masks [128,Ttot*SPAN] bf16,
             idxs [16, L//16] i16, coefb [128,M] f32, ident [128,128] f32
    Output : out [RPC, D] f32 (own slice of acc)
    """
    nc = bacc.Bacc(None, target_bir_lowering=False, debug=False,
                   num_swdge_queues=NQ)

    xfull_e = nc.declare_dram_parameter("xfull", [N, D], BF16, isOutput=False)
    x0T_e = nc.declare_dram_parameter("x0T", [128, RPC], F32, isOutput=False)
    masks_e = nc.declare_dram_parameter("masks", [128, MW], BF16, isOutput=False)
    idxs_e = nc.declare_dram_parameter("idxs", [128, L // 16], I16, isOutput=False)
    coefb_e = nc.declare_dram_parameter("coefb", [128, M], F32, isOutput=False)
    ident_e = nc.declare_dram_parameter("ident", [128, 128], F32, isOutput=False)
    out_e = nc.declare_dram_parameter("out", [RPC, D], F32, isOutput=True)

    Xbuf = nc.dram_tensor("Xbuf", [N, D], BF16,
                          addr_space="Local" if _LOCAL_XBUF else "Shared")
    slice_hbm = nc.dram_tensor("slice_hbm", [RPC, D], BF16)

    with tile.TileContext(nc) as tc:
        with (
            tc.tile_pool(name="const", bufs=1) as cpool,
            tc.tile_pool(name="state", bufs=1) as spool,
            tc.tile_pool(name="g", bufs=GBUFS) as gpool,
            tc.tile_pool(name="stage", bufs=2) as stpool,
            tc.tile_pool(name="psw", bufs=2, space="PSUM") as pswpool,
            tc.tile_pool(name="pst", bufs=2, space="PSUM") as pstpool,
        ):
            masks_sb = cpool.tile([128, MW], BF16)
            idxs_sb = cpool.tile([128, L // 16], I16)
            coefb_sb = cpool.tile([128, M], F32)
            ident_sb = cpool.tile([128, 128], F32)

            slabs = [spool.tile([128, RPC], F32, tag=f"slab{i}", name=f"slab{i}") for i in range(2)]
            acc = spool.tile([128, RPC], F32, tag="acc")

            # ---- prologue: load constants + init state ----
            nc.sync.dma_start(out=masks_sb[:, :], in_=masks_e[:, :])
            nc.sync.dma_start(out=idxs_sb[:, :], in_=idxs_e[:, :])
            nc.sync.dma_start(out=coefb_sb[:, :], in_=coefb_e[:, :])
            nc.sync.dma_start(out=ident_sb[:, :], in_=ident_e[:, :])
            nc.sync.dma_start(out=slabs[0][:, :], in_=x0T_e[:, :])
            nc.gpsimd.dma_start(out=Xbuf[:, :], in_=xfull_e[:, :])
            # acc = c0 * T0
            nc.vector.tensor_scalar(
                acc[:, :], slabs[0][:, :], coefb_sb[:, 0:1], None, ALU.mult
            )

            def emit_windows(k, Tprev2, Tout, do_store):
                """One SpMM pass + per-window epilogue.
                do_store: store transposed T_k rows to slice_hbm (AllGather'd
                into Xbuf after the last window).
                """
                qi = 0
                for w in range(NW):
                    wlo = w * W
                    wsize = min(W, RPC - wlo)
                    psw = pswpool.tile([128, W], F32)
                    nc.vector.memset(psw[:, :wsize], 0.0)
                    for s in (0, 1):
                        seg = shared_segs[2 * w + s]
                        base = 0 if s == 0 else COLSPLIT
                        src = Xbuf[base:base + COLSPLIT, :]
                        nt_all = seg["ntiles"]
                        for c0 in range(0, nt_all, GCHUNK):
                            nt = min(GCHUNK, nt_all - c0)
                            G = gpool.tile([128, GCHUNK, TILE], BF16, tag="g", name="gt")
                            off = seg["idx_off"] + c0 * TILE
                            if not _SKIP_GATHER:
                                nc.gpsimd.dma_gather(
                                    G[:, :nt, :],
                                    src,
                                    idxs_sb[:, off // 16: off // 16 + nt * TILE // 16],
                                    nt * TILE,
                                    nt * TILE,
                                    TILE,
                                    single_packet=False,
                                    queue_num=qi % NQ,
                                )
                                qi += 1
                            for t in range(nt):
                                mo = seg["moff0"] + (c0 + t) * seg["mstride"]
                                r0 = seg["r0s"][c0 + t]
                                span = seg["spans"][c0 + t]
                                is_last = (
                                    s == 1 and c0 + nt == nt_all and t == nt - 1
                                )
                                if not _SKIP_MM:
                                    nc.tensor.matmul(
                                        psw[:, r0:r0 + span],
                                        G[:, t, :],
                                        masks_sb[:, mo: mo + span],
                                        start=False,
                                        stop=is_last,
                                        skip_group_check=True,
                                    )
                    # window epilogue
                    if _SKIP_EPI:
                        continue
                    if k == 1:
                        nc.vector.tensor_scalar(
                            Tout[:, wlo:wlo + wsize], psw[:, :wsize],
                            0.5, None, ALU.mult,
                        )
                    else:
                        nc.vector.scalar_tensor_tensor(
                            Tout[:, wlo:wlo + wsize], psw[:, :wsize], 1.0,
                            Tprev2[:, wlo:wlo + wsize], ALU.mult, ALU.subtract,
                        )
                    nc.vector.scalar_tensor_tensor(
                        acc[:, wlo:wlo + wsize], Tout[:, wlo:wlo + wsize],
                        coefb_sb[:, k:k + 1], acc[:, wlo:wlo + wsize],
                        ALU.mult, ALU.add,
                    )
                    if do_store:
                        emit_transposed_store(Tout, wlo, wsize, slice_hbm,
                                              wlo, dt=BF16)
                        if not _SKIP_AG and w == NW - 1:
                            nc.gpsimd.collective_compute(
                                "AllGather", ALU.bypass,
                                replica_groups=[list(range(CORES))],
                                ins=[slice_hbm.ap().opt()],
                                outs=[Xbuf.ap().opt()],
                            )

            def emit_transposed_store(srcT, wlo, wsize, dest, dlo, dt=F32):
                """dest[dlo:dlo+wsize, :] = srcT[:, wlo:wlo+wsize].T"""
                nq = (wsize + 127) // 128
                pst = pstpool.tile([128, W], F32)
                stage = stpool.tile([128, W], dt)
                for q in range(nq):
                    qsz = min(128, wsize - q * 128)
                    nc.tensor.transpose(
                        pst[:qsz, q * 128:q * 128 + 128],
                        srcT[:, wlo + q * 128: wlo + q * 128 + qsz],
                        ident_sb[:, :],
                    )
                if wsize % 128 == 0:
                    nc.vector.tensor_copy(stage[:, :nq * 128], pst[:, :nq * 128])
                else:
                    nc.vector.tensor_copy(
                        stage[:wsize, :nq * 128], pst[:wsize, :nq * 128]
                    )
                if wsize % 128 == 0:
                    # dest row (dlo + q*128 + p) <- stage[p, q*128 + f]
                    dest_ap = dest[dlo:dlo + wsize, :].rearrange(
                        "(q p) f -> p q f", p=128
                    )
                    nc.sync.dma_start(out=dest_ap, in_=stage[:, :nq * 128])
                else:
                    # last partial window: single q, partial partitions
                    assert nq == 1
                    nc.sync.dma_start(
                        out=dest[dlo:dlo + wsize, :], in_=stage[:wsize, :D]
                    )

            for k in range(1, n_iters + 1):
                # In-place recurrence: T_k overwrites T_{k-2}'s slab; the
                # epilogue reads Tprev2[w] and writes Tout[w] elementwise.
                Tprev2 = slabs[k % 2]
                Tout = slabs[k % 2]
                emit_windows(k, Tprev2, Tout, do_store=k < n_iters)

            # epilogue: out = acc.T
            for w in range(NW):
                wlo = w * W
                wsize = min(W, RPC - wlo)
                emit_transposed_store(acc, wlo, wsize, out_e, wlo)

    return nc


def _make_in_maps(X, coeffs, per_core):
    bfnp = mybir.dt.np(BF16)
    ident = np.eye(128, dtype=np.float32)
    coefb = np.broadcast_to(np.asarray(coeffs, np.float32)[None, :], (128, M)).copy()
    X_bf = np.ascontiguousarray(X).astype(bfnp)
    in_maps = []
    for c in range(CORES):
        x0T = np.ascontiguousarray(X[c * RPC:(c + 1) * RPC].T)
        in_maps.append({
            "xfull": X_bf,
            "x0T": x0T,
            "masks": per_core[c]["masks"].astype(bfnp),
            "idxs": per_core[c]["idxs"],
            "coefb": coefb,
            "ident": ident,
        })
    return in_maps


def kernel(rows, cols, vals, X, coeffs, _trace=False):
    rows = np.asarray(rows)
    cols = np.asarray(cols)
    vals = np.asarray(vals, np.float32)
    X = np.asarray(X, np.float32)
    coeffs = np.asarray(coeffs, np.float32)

    shared_segs, per_core, Ttot, L, MW = build_plans(rows, cols, vals)
    nc = build_kernel(shared_segs, Ttot, L, MW, n_iters=M - 1)
    nc.compile()
    in_maps = _make_in_maps(X, coeffs, per_core)
    res = run_bass_kernel_spmd(nc, in_maps, list(range(CORES)), trace=_trace)
    out = np.concatenate([res.results[c]["out"] for c in range(CORES)], axis=0)
    if _trace:
        kernel.last_results = res
    return out


def bench_exec_ns(rows, cols, vals, X, coeffs, reps=5):
    """Time on-device execution with device-resident inputs (excludes
    compile and H2D of the real inputs; fresh donated output buffers are
    staged untimed before each rep)."""
    import time
    import jax
    from jax.sharding import Mesh, PartitionSpec
    from jax.experimental.shard_map import shard_map
    from concourse import bass2jax
    from concourse.bass2jax import _bass_exec_p, partition_id_tensor
    import concourse.mybir as _mb

    rows = np.asarray(rows); cols = np.asarray(cols)
    vals = np.asarray(vals, np.float32); X = np.asarray(X, np.float32)
    coeffs = np.asarray(coeffs, np.float32)
    shared_segs, per_core, Ttot, L, MW = build_plans(rows, cols, vals)
    nc = build_kernel(shared_segs, Ttot, L, MW, n_iters=M - 1)
    nc.compile()
    in_maps = _make_in_maps(X, coeffs, per_core)

    bass2jax.install_neuronx_cc_hook()
    partition_name = nc.partition_id_tensor.name if nc.partition_id_tensor else None
    in_names, out_names, out_avals, zero_outs = [], [], [], []
    for alloc in nc.m.functions[0].allocations:
        if not isinstance(alloc, _mb.MemoryLocationSet):
            continue
        name = alloc.memorylocations[0].name
        if alloc.kind == "ExternalInput":
            if name != partition_name:
                in_names.append(name)
        elif alloc.kind == "ExternalOutput":
            out_names.append(name)
            shape = tuple(alloc.tensor_shape)
            dtype = _mb.dt.np(alloc.dtype)
            out_avals.append(jax.core.ShapedArray(shape, dtype))
            zero_outs.append(np.zeros(shape, dtype))
    n_params = len(in_names)
    n_outs = len(out_avals)
    in_names.extend(out_names)
    if partition_name is not None:
        in_names.append(partition_name)
    donate = tuple(range(n_params, n_params + n_outs))

    def _body(*args):
        operands = list(args)
        if partition_name is not None:
            operands.append(partition_id_tensor())
        return tuple(_bass_exec_p.bind(
            *operands, out_avals=tuple(out_avals), in_names=tuple(in_names),
            out_names=tuple(out_names), lowering_input_output_aliases=(),
            sim_require_finite=False, sim_require_nnan=False, nc=nc))

    devices = jax.devices()[:CORES]
    mesh = Mesh(np.asarray(devices), ("core",))
    sharded = jax.jit(
        shard_map(_body, mesh=mesh,
                  in_specs=(PartitionSpec("core"),) * (n_params + n_outs),
                  out_specs=(PartitionSpec("core"),) * n_outs,
                  check_rep=False),
        donate_argnums=donate, keep_unused=True)
    per_core_in = [[np.asarray(m[nm]) for nm in in_names[:n_params]] for m in in_maps]
    concat_in = [np.concatenate([per_core_in[c][i] for c in range(CORES)], axis=0)
                 for i in range(n_params)]
    sharding = jax.sharding.NamedSharding(mesh, PartitionSpec("core"))
    dev_in = [jax.device_put(a, sharding) for a in concat_in]

    def fresh_zeros():
        return [jax.device_put(
            np.zeros((CORES * z.shape[0], *z.shape[1:]), z.dtype), sharding)
            for z in zero_outs]

    # warmup (compiles)
    outs = sharded(*dev_in, *fresh_zeros())
    jax.block_until_ready(outs)
    times = []
    for _ in range(reps):
        zs = fresh_zeros()
        jax.block_until_ready(zs)
        t0 = time.perf_counter()
        outs = sharded(*dev_in, *zs)
        jax.block_until_ready(outs)
        times.append(time.perf_counter() - t0)
    return int(min(times) * 1e9), [int(t * 1e9) for t in times]



# revision 53
# speedup vs baseline: 1.3546x; 1.0470x over previous
"""Distributed Chebyshev SpMM kernel for 8 Trainium2 NeuronCores.

acc = sum_k coeffs[k] * T_k(L) @ X with T_k = 2 L T_{k-1} - T_{k-2} over a
sparse 50000-node / 800000-edge graph, feature dim 128, 30 coefficients.

Strategy: row-shard nodes across 8 cores. Per Chebyshev step each core
dma_gathers T_{k-1}[col] rows (bf16, 256B) for its ~100K edges from an HBM
copy of the full T_{k-1}, spread over all 4 SWDGE queues (the per-queue
descriptor ring is the gather bottleneck), and segment-reduces on the
TensorEngine: gathered tile is the bf16 stationary operand, a
host-precomputed 2*val*onehot bf16 mask is the moving operand, accumulating
fp32 output rows in PSUM. The recurrence/accumulator math stays fp32 in
SBUF (in-place two-slab update).

The new T_k slice is transposed-stored to HBM per window and one
AllGather per iteration rebuilds the full bf16 gather source (Shared
address space for the fast collective path). Masks/indices are
iteration-invariant and loaded to SBUF once; gather indices are int16,
so edges are split into two col-range streams (<25000 / >=25000).
"""
import sys
sys.path.insert(0, "/opt/trn_rl_repo")
import os
import numpy as np

_SKIP_AG = bool(int(os.environ.get("K_SKIP_AG", "0")))
_SKIP_GATHER = bool(int(os.environ.get("K_SKIP_GATHER", "0")))
_SKIP_MM = bool(int(os.environ.get("K_SKIP_MM", "0")))
_SKIP_EPI = bool(int(os.environ.get("K_SKIP_EPI", "0")))
_LOCAL_XBUF = bool(int(os.environ.get("K_LOCAL_XBUF", "0")))


N = 50000
D = 128
NNZ = 800000
M = 30
CORES = 8
RPC = N // CORES          # 6250
W = 512
NW = (RPC + W - 1) // W   # 13 (12x512 + 106)
TILE = 128
_SPAN = int(os.environ.get("K_SPAN", "24"))
SPAN_S = (_SPAN, _SPAN)  # packer span cap per segment kind (96% tile fill)
# Edges are bucketed by global source column half (col < 25000): gather
# indices stay within int16 range and each half is a contiguous row range
# of the single AllGather'd Xbuf.
COLSPLIT = 25000


def build_plans(rows, cols, vals):
    """Returns (shared_segs, per_core) where
    shared_segs: list over segments of dict(w, s, ntiles, idx_off,
                 r0s[ntiles], spans[ntiles])
    per_core: list of dict(idxs int16 [16, L/16], masks f32 [128, Ttot*SPAN])
    """
    rows = np.asarray(rows).astype(np.int64)
    cols = np.asarray(cols).astype(np.int64)
    vals = np.asarray(vals).astype(np.float32)

    # per-core sorted edge lists per (w, s); s buckets by global col half,
    # cols pre-mapped to gather indices into the matching Xbuf half.
    core_seg_edges = [[] for _ in range(CORES)]  # [(er, ec, ev)] per segment
    for c in range(CORES):
        r0c = c * RPC
        sel = (rows >= r0c) & (rows < r0c + RPC)
        er_all = rows[sel] - r0c
        ec_all = cols[sel]
        ev_all = vals[sel]
        s_all = ec_all >= COLSPLIT
        gi_all = np.where(s_all, ec_all - COLSPLIT, ec_all)
        for w in range(NW):
            rlo = w * W
            rhi = min(rlo + W, RPC)
            inw = (er_all >= rlo) & (er_all < rhi)
            for s in range(2):
                m = inw & (s_all == bool(s))
                er = er_all[m] - rlo
                ec = gi_all[m]
                ev = ev_all[m]
                o = np.argsort(er, kind="stable")
                core_seg_edges[c].append((er[o], ec[o], ev[o]))

    nseg = NW * 2
    shared_segs = []
    per_core_tiles = [[] for _ in range(CORES)]  # (idx128, rw128, val128) per tile
    tile_moffs = []  # mask column offset per global tile
    idx_off = 0
    moff = 0
    for si in range(nseg):
        w, s = divmod(si, 2)
        wsize = min(W, RPC - w * W)
        span_cap = SPAN_S[s]
        # Joint greedy schedule: r0_t = min over cores of next pending row;
        # each core then takes up to 128 edges with rows < r0_t + span.
        # Feasible by construction for every core.
        segs_e = [core_seg_edges[c][si] for c in range(CORES)]
        pos = [0] * CORES
        nes = [len(e[0]) for e in segs_e]
        r0s, spans = [], []
        takes = []  # per tile: list of (core_pos, take)
        prev = 0
        while any(pos[c] < nes[c] for c in range(CORES)):
            nextrow = min(
                (int(segs_e[c][0][pos[c]]) for c in range(CORES)
                 if pos[c] < nes[c]),
            )
            r0 = max(prev, min(nextrow, max(0, wsize - 1)))
            span = min(span_cap, wsize - r0)
            tile_takes = []
            for c in range(CORES):
                er = segs_e[c][0]
                hi = np.searchsorted(er, r0 + span)
                take = int(min(TILE, hi - pos[c]))
                take = max(0, take)
                tile_takes.append((pos[c], take))
                pos[c] += take
            r0s.append(r0)
            spans.append(span)
            takes.append(tile_takes)
            prev = r0
            assert len(r0s) < 96, (si, len(r0s))
        ntiles = len(r0s)

        # pack each core
        for c in range(CORES):
            er, ec, ev = segs_e[c]
            for t in range(ntiles):
                r0, span = r0s[t], spans[t]
                p0, take = takes[t][c]
                idx_t = np.zeros(TILE, np.int64)
                rw_t = np.full(TILE, r0, np.int64)
                val_t = np.zeros(TILE, np.float32)
                if take > 0:
                    idx_t[:take] = ec[p0:p0 + take]
                    rw_t[:take] = er[p0:p0 + take]
                    val_t[:take] = 2.0 * ev[p0:p0 + take]
                    assert er[p0] >= r0, (c, si, t, er[p0], r0)
                    assert er[p0 + take - 1] < r0 + span
                per_core_tiles[c].append((idx_t, rw_t - r0, val_t))
            assert pos[c] == len(er), (c, si, pos[c], len(er))

        shared_segs.append(dict(w=w, s=s, ntiles=ntiles, idx_off=idx_off,
                                r0s=r0s, spans=spans, moff0=moff,
                                mstride=span_cap))
        tile_moffs.extend(moff + t * span_cap for t in range(ntiles))
        idx_off += ntiles * TILE
        moff += ntiles * span_cap

    L = idx_off
    MW = moff
    Ttot = L // TILE
    per_core = []
    for c in range(CORES):
        tiles = per_core_tiles[c]
        idx_flat = np.concatenate([t[0] for t in tiles])
        masks = np.zeros((TILE, MW), np.float32)
        for g, (idx_t, loc_t, val_t) in enumerate(tiles):
            masks[np.arange(TILE), tile_moffs[g] + loc_t] = val_t
        idxs = np.ascontiguousarray(np.tile(idx_flat.reshape(L // 16, 16).T.astype(np.int16), (8, 1)))
        per_core.append(dict(idxs=idxs, masks=masks))
    return shared_segs, per_core, Ttot, L, MW


def sim_core_spmm(shared_segs, core_data, xb):
    """Numpy sim of one SpMM: returns [128, RPC] feat-major = rows of 2*L@X.
    xb = (X[:25000], X[25000:]): the two gather-source halves."""
    out = np.zeros((D, RPC), np.float32)
    idxs = core_data["idxs"]
    masks = core_data["masks"]
    for seg in shared_segs:
        src = xb[seg["s"]]
        Lseg = seg["ntiles"] * TILE
        off = seg["idx_off"]
        j = np.arange(Lseg)
        unwrapped = idxs[(off + j) % 16, (off + j) // 16].astype(np.int64)
        G = src[unwrapped]
        for t in range(seg["ntiles"]):
            Gt = G[t * TILE:(t + 1) * TILE]
            r0, span = seg["r0s"][t], seg["spans"][t]
            mo = seg["moff0"] + t * seg["mstride"]
            mk = masks[:, mo: mo + span]
            out[:, seg["w"] * W + r0: seg["w"] * W + r0 + span] += Gt.T @ mk
    return out


from concourse import bass, mybir, bacc
from concourse import tile
from concourse.bass_utils import run_bass_kernel_spmd

F32 = mybir.dt.float32
BF16 = mybir.dt.bfloat16
I16 = mybir.dt.int16
ALU = mybir.AluOpType

GCHUNK = int(os.environ.get("K_GCHUNK", "18"))  # tiles per gather call
NQ = int(os.environ.get("K_NQ", "4"))      # SWDGE gather queues
GBUFS = int(os.environ.get("K_GBUFS", "8"))  # gather tile pool depth


def build_kernel(shared_segs, Ttot, L, MW, n_iters=M - 1):
    """One shared SPMD program; per-core variation via inputs only.

    Inputs : xfull [N, D] bf16,
             x0T [128,RPC] f32, masks [128,Ttot*SPAN] bf16,
             idxs [16, L//16] i16, coefb [128,M] f32, ident [128,128] f32
    Output : out [RPC, D] f32 (own slice of acc)
    """
    nc = bacc.Bacc(None, target_bir_lowering=False, debug=False,
                   num_swdge_queues=NQ)

    xfull_e = nc.declare_dram_parameter("xfull", [N, D], BF16, isOutput=False)
    x0T_e = nc.declare_dram_parameter("x0T", [128, RPC], F32, isOutput=False)
    masks_e = nc.declare_dram_parameter("masks", [128, MW], BF16, isOutput=False)
    idxs_e = nc.declare_dram_parameter("idxs", [128, L // 16], I16, isOutput=False)
    coefb_e = nc.declare_dram_parameter("coefb", [128, M], F32, isOutput=False)
    ident_e = nc.declare_dram_parameter("ident", [128, 128], F32, isOutput=False)
    out_e = nc.declare_dram_parameter("out", [RPC, D], F32, isOutput=True)

    Xbuf = nc.dram_tensor("Xbuf", [N, D], BF16,
                          addr_space="Local" if _LOCAL_XBUF else "Shared")
    slice_hbm = nc.dram_tensor("slice_hbm", [RPC, D], BF16)

    with tile.TileContext(nc) as tc:
        with (
            tc.tile_pool(name="const", bufs=1) as cpool,
            tc.tile_pool(name="state", bufs=1) as spool,
            tc.tile_pool(name="g", bufs=GBUFS) as gpool,
            tc.tile_pool(name="stage", bufs=2) as stpool,
            tc.tile_pool(name="psw", bufs=2, space="PSUM") as pswpool,
            tc.tile_pool(name="pst", bufs=2, space="PSUM") as pstpool,
        ):
            masks_sb = cpool.tile([128, MW], BF16)
            idxs_sb = cpool.tile([128, L // 16], I16)
            coefb_sb = cpool.tile([128, M], F32)
            ident_sb = cpool.tile([128, 128], F32)

            slabs = [spool.tile([128, RPC], F32, tag=f"slab{i}", name=f"slab{i}") for i in range(2)]
            acc = spool.tile([128, RPC], F32, tag="acc")

            # ---- prologue: load constants + init state ----
            nc.sync.dma_start(out=masks_sb[:, :], in_=masks_e[:, :])
            nc.sync.dma_start(out=idxs_sb[:, :], in_=idxs_e[:, :])
            nc.sync.dma_start(out=coefb_sb[:, :], in_=coefb_e[:, :])
            nc.sync.dma_start(out=ident_sb[:, :], in_=ident_e[:, :])
            nc.sync.dma_start(out=slabs[0][:, :], in_=x0T_e[:, :])
            nc.gpsimd.dma_start(out=Xbuf[:, :], in_=xfull_e[:, :])
            # acc = c0 * T0
            nc.vector.tensor_scalar(
                acc[:, :], slabs[0][:, :], coefb_sb[:, 0:1], None, ALU.mult
            )

            def emit_windows(k, Tprev2, Tout, do_store):
                """One SpMM pass + per-window epilogue.
                do_store: store transposed T_k rows to slice_hbm (AllGather'd
                into Xbuf after the last window).
                """
                qi = 0
                for w in range(NW):
                    wlo = w * W
                    wsize = min(W, RPC - wlo)
                    psw = pswpool.tile([128, W], F32)
                    nc.vector.memset(psw[:, :wsize], 0.0)
                    for s in (0, 1):
                        seg = shared_segs[2 * w + s]
                        base = 0 if s == 0 else COLSPLIT
                        src = Xbuf[base:base + COLSPLIT, :]
                        nt_all = seg["ntiles"]
                        for c0 in range(0, nt_all, GCHUNK):
                            nt = min(GCHUNK, nt_all - c0)
                            G = gpool.tile([128, GCHUNK, TILE], BF16, tag="g", name="gt")
                            off = seg["idx_off"] + c0 * TILE
                            if not _SKIP_GATHER:
                                nc.gpsimd.dma_gather(
                                    G[:, :nt, :],
                                    src,
                                    idxs_sb[:, off // 16: off // 16 + nt * TILE // 16],
                                    nt * TILE,
                                    nt * TILE,
                                    TILE,
                                    single_packet=False,
                                    queue_num=qi % NQ,
                                )
                                qi += 1
                            for t in range(nt):
                                mo = seg["moff0"] + (c0 + t) * seg["mstride"]
                                r0 = seg["r0s"][c0 + t]
                                span = seg["spans"][c0 + t]
                                is_last = (
                                    s == 1 and c0 + nt == nt_all and t == nt - 1
                                )
                                if not _SKIP_MM:
                                    nc.tensor.matmul(
                                        psw[:, r0:r0 + span],
                                        G[:, t, :],
                                        masks_sb[:, mo: mo + span],
                                        start=False,
                                        stop=is_last,
                                        skip_group_check=True,
                                    )
                    # window epilogue
                    if _SKIP_EPI:
                        continue
                    if k == 1:
                        nc.vector.tensor_scalar(
                            Tout[:, wlo:wlo + wsize], psw[:, :wsize],
                            0.5, None, ALU.mult,
                        )
                    else:
                        nc.vector.scalar_tensor_tensor(
                            Tout[:, wlo:wlo + wsize], psw[:, :wsize], 1.0,
                            Tprev2[:, wlo:wlo + wsize], ALU.mult, ALU.subtract,
                        )
                    nc.vector.scalar_tensor_tensor(
                        acc[:, wlo:wlo + wsize], Tout[:, wlo:wlo + wsize],
                        coefb_sb[:, k:k + 1], acc[:, wlo:wlo + wsize],
                        ALU.mult, ALU.add,
                    )
                    if do_store:
                        emit_transposed_store(Tout, wlo, wsize, slice_hbm,
                                              wlo, dt=BF16)
                        if not _SKIP_AG and w == NW - 1:
                            nc.gpsimd.collective_compute(
                                "AllGather", ALU.bypass,
                                replica_groups=[list(range(CORES))],
                                ins=[slice_hbm.ap().opt()],
                                outs=[Xbuf.ap().opt()],
                            )

            def emit_transposed_store(srcT, wlo, wsize, dest, dlo, dt=F32):
                """dest[dlo:dlo+wsize, :] = srcT[:, wlo:wlo+wsize].T"""
                nq = (wsize + 127) // 128
                pst = pstpool.tile([128, W], F32)
                stage = stpool.tile([128, W], dt)
                for q in range(nq):
                    qsz = min(128, wsize - q * 128)
                    nc.tensor.transpose(
                        pst[:qsz, q * 128:q * 128 + 128],
                        srcT[:, wlo + q * 128: wlo + q * 128 + qsz],
                        ident_sb[:, :],
                    )
                if wsize % 128 == 0:
                    nc.vector.tensor_copy(stage[:, :nq * 128], pst[:, :nq * 128])
                else:
                    nc.vector.tensor_copy(
                        stage[:wsize, :nq * 128], pst[:wsize, :nq * 128]
                    )
                if wsize % 128 == 0:
                    # dest row (dlo + q*128 + p) <- stage[p, q*128 + f]
                    dest_ap = dest[dlo:dlo + wsize, :].rearrange(
                        "(q p) f -> p q f", p=128
                    )
                    nc.sync.dma_start(out=dest_ap, in_=stage[:, :nq * 128])
                else:
                    # last partial window: single q, partial partitions
                    assert nq == 1
                    nc.sync.dma_start(
                        out=dest[dlo:dlo + wsize, :], in_=stage[:wsize, :D]
                    )

            for k in range(1, n_iters + 1):
                # In-place recurrence: T_k overwrites T_{k-2}'s slab; the
                # epilogue reads Tprev2[w] and writes Tout[w] elementwise.
                Tprev2 = slabs[k % 2]
                Tout = slabs[k % 2]
                emit_windows(k, Tprev2, Tout, do_store=k < n_iters)

            # epilogue: out = acc.T
            for w in range(NW):
                wlo = w * W
                wsize = min(W, RPC - wlo)
                emit_transposed_store(acc, wlo, wsize, out_e, wlo)

    return nc


def _make_in_maps(X, coeffs, per_core):
    bfnp = mybir.dt.np(BF16)
    ident = np.eye(128, dtype=np.float32)
    coefb = np.broadcast_to(np.asarray(coeffs, np.float32)[None, :], (128, M)).copy()
    X_bf = np.ascontiguousarray(X).astype(bfnp)
    in_maps = []
    for c in range(CORES):
        x0T = np.ascontiguousarray(X[c * RPC:(c + 1) * RPC].T)
        in_maps.append({
            "xfull": X_bf,
            "x0T": x0T,
            "masks": per_core[c]["masks"].astype(bfnp),
            "idxs": per_core[c]["idxs"],
            "coefb": coefb,
            "ident": ident,
        })
    return in_maps


def kernel(rows, cols, vals, X, coeffs, _trace=False):
    rows = np.asarray(rows)
    cols = np.asarray(cols)
    vals = np.asarray(vals, np.float32)
    X = np.asarray(X, np.float32)
    coeffs = np.asarray(coeffs, np.float32)

    shared_segs, per_core, Ttot, L, MW = build_plans(rows, cols, vals)
    nc = build_kernel(shared_segs, Ttot, L, MW, n_iters=M - 1)
    nc.compile()
    in_maps = _make_in_maps(X, coeffs, per_core)
    res = run_bass_kernel_spmd(nc, in_maps, list(range(CORES)), trace=_trace)
    out = np.concatenate([res.results[c]["out"] for c in range(CORES)], axis=0)
    if _trace:
        kernel.last_results = res
    return out


def bench_exec_ns(rows, cols, vals, X, coeffs, reps=5):
    """Time on-device execution with device-resident inputs (excludes
    compile and H2D of the real inputs; fresh donated output buffers are
    staged untimed before each rep)."""
    import time
    import jax
    from jax.sharding import Mesh, PartitionSpec
    from jax.experimental.shard_map import shard_map
    from concourse import bass2jax
    from concourse.bass2jax import _bass_exec_p, partition_id_tensor
    import concourse.mybir as _mb

    rows = np.asarray(rows); cols = np.asarray(cols)
    vals = np.asarray(vals, np.float32); X = np.asarray(X, np.float32)
    coeffs = np.asarray(coeffs, np.float32)
    shared_segs, per_core, Ttot, L, MW = build_plans(rows, cols, vals)
    nc = build_kernel(shared_segs, Ttot, L, MW, n_iters=M - 1)
    nc.compile()
    in_maps = _make_in_maps(X, coeffs, per_core)

    bass2jax.install_neuronx_cc_hook()
    partition_name = nc.partition_id_tensor.name if nc.partition_id_tensor else None
    in_names, out_names, out_avals, zero_outs = [], [], [], []
    for alloc in nc.m.functions[0].allocations:
        if not isinstance(alloc, _mb.MemoryLocationSet):
            continue
        name = alloc.memorylocations[0].name
        if alloc.kind == "ExternalInput":
            if name != partition_name:
                in_names.append(name)
        elif alloc.kind == "ExternalOutput":
            out_names.append(name)
            shape = tuple(alloc.tensor_shape)
            dtype = _mb.dt.np(alloc.dtype)
            out_avals.append(jax.core.ShapedArray(shape, dtype))
            zero_outs.append(np.zeros(shape, dtype))
    n_params = len(in_names)
    n_outs = len(out_avals)
    in_names.extend(out_names)
    if partition_name is not None:
        in_names.append(partition_name)
    donate = tuple(range(n_params, n_params + n_outs))

    def _body(*args):
        operands = list(args)
        if partition_name is not None:
            operands.append(partition_id_tensor())
        return tuple(_bass_exec_p.bind(
            *operands, out_avals=tuple(out_avals), in_names=tuple(in_names),
            out_names=tuple(out_names), lowering_input_output_aliases=(),
            sim_require_finite=False, sim_require_nnan=False, nc=nc))

    devices = jax.devices()[:CORES]
    mesh = Mesh(np.asarray(devices), ("core",))
    sharded = jax.jit(
        shard_map(_body, mesh=mesh,
                  in_specs=(PartitionSpec("core"),) * (n_params + n_outs),
                  out_specs=(PartitionSpec("core"),) * n_outs,
                  check_rep=False),
        donate_argnums=donate, keep_unused=True)
    per_core_in = [[np.asarray(m[nm]) for nm in in_names[:n_params]] for m in in_maps]
    concat_in = [np.concatenate([per_core_in[c][i] for c in range(CORES)], axis=0)
                 for i in range(n_params)]
    sharding = jax.sharding.NamedSharding(mesh, PartitionSpec("core"))
    dev_in = [jax.device_put(a, sharding) for a in concat_in]

    def fresh_zeros():
        return [jax.device_put(
            np.zeros((CORES * z.shape[0], *z.shape[1:]), z.dtype), sharding)
            for z in zero_outs]

    # warmup (compiles)
    outs = sharded(*dev_in, *fresh_zeros())
    jax.block_until_ready(outs)
    times = []
    for _ in range(reps):
        zs = fresh_zeros()
        jax.block_until_ready(zs)
        t0 = time.perf_counter()
        outs = sharded(*dev_in, *zs)
        jax.block_until_ready(outs)
        times.append(time.perf_counter() - t0)
    return int(min(times) * 1e9), [int(t * 1e9) for t in times]

